# revision 5
# baseline (speedup 1.0000x reference)
"""Trainium2 Bass kernel for 2-layer GCN + mean-pool + GRU step + layernorm + linear.

Strategy (8 NeuronCores, SPMD single program, per-core data):
- Graph-aligned node sharding: core c owns graphs [256c, 256(c+1)) and their
  (contiguous, batch is sorted) node range. Per-core nodes padded to 12544.
- Both GCN layers commute the weight matmul past aggregation:
      h = relu(Agg(x) @ W + b),  Agg = D^-1/2 (A+I) D^-1/2
  so the gather reads raw features; self-loops are regular edges; the
  symmetric norm is dinv[src] (per-edge stream) times dinv[dst] (folded into
  the post-matmul activation scale).
- Gather: dma_gather (int16 idx, 4 table windows of 25088 rows = 2 core
  blocks each). Node-major K-slot layout: tile of 128 nodes gathers K
  neighbor rows per node; one DVE bcast-multiply (by dinv[src]) and one
  strided DVE reduce produce the aggregate.
- One AllGather of h1 between layers. Pooling via per-tile matmuls against a
  host-built [node,graph] selector with 1/cnt folded in. Head (GRU+LN+linear)
  computed per-core on its 256 graphs in feature-major layout.
"""

import numpy as np

N_NODES = 100000
N_EDGES = 1600000
N_GRAPHS = 2048
D = 128
P = 128
EPS = 1e-5
NC = 8
GPC = N_GRAPHS // NC          # graphs per core = 256
NPC = 13056                   # padded nodes per core (102 * 128)
TILES = NPC // P              # 102
TBL = NC * NPC                # global padded table rows
NBANK = 4
BANK = TBL // NBANK           # rows per dma_gather window (< 32768)
MAX_IDX_PER_CALL = 1024

_CACHE = {}
_last_in_maps = None


def _prep(x, src, dst, batch):
    """Host-side graph-structure preprocessing (indices/degrees only)."""
    deg = np.bincount(dst, minlength=N_NODES).astype(np.float64) + 1.0
    dinv = (1.0 / np.sqrt(deg)).astype(np.float32)

    # graph-aligned shard boundaries
    node_start = np.searchsorted(batch, np.arange(0, N_GRAPHS + 1, GPC))
    core_of = np.searchsorted(node_start, np.arange(N_NODES), side="right") - 1

    # per-edge core/bank info (self-loops appended: one per node)
    a_src = np.concatenate([src, np.arange(N_NODES)])
    a_dst = np.concatenate([dst, np.arange(N_NODES)])
    e_core = core_of[a_dst]
    s_bank = core_of[a_src] // 2  # bank = 2 consecutive core blocks

    # per (core, local node) bank counts -> per-core perm (profile sort)
    perm_pos = np.zeros(N_NODES, np.int64)
    percore = []
    for c in range(NC):
        lo, hi = node_start[c], node_start[c + 1]
        nloc = hi - lo
        assert nloc <= NPC, f"core {c} has {nloc} nodes > NPC={NPC}"
        m = (a_dst >= lo) & (a_dst < hi)
        ldst = a_dst[m] - lo
        lbank = s_bank[m]
        kb = np.zeros((nloc, NBANK), np.int64)
        np.add.at(kb, (ldst, lbank), 1)
        order = np.lexsort((-kb[:, 3], -kb[:, 2], -kb[:, 1], -kb[:, 0]))
        pos = np.empty(nloc, np.int64)
        pos[order] = np.arange(nloc)
        perm_pos[lo:hi] = pos
        percore.append((lo, hi, m, kb, pos))

    pid = core_of * NPC + perm_pos  # global permuted id of each orig node

    # shared schedule: K_b per (tile, bank) = max over cores of per-core max count
    ksched = np.zeros((TILES, NBANK), np.int64)
    for c in range(NC):
        lo, hi, m, kb, pos = percore[c]
        nloc = hi - lo
        tile_of = pos // P
        for b in range(NBANK):
            kmax = np.zeros(TILES, np.int64)
            np.maximum.at(kmax, tile_of, kb[:, b])
            ksched[:, b] = np.maximum(ksched[:, b], kmax)
    ksched = np.maximum(ksched, 1)
    kt = ksched.sum(axis=1)                       # total slots per tile
    tile_base = np.concatenate([[0], np.cumsum(kt)])[:-1]
    bank_base = np.concatenate([np.zeros((TILES, 1), np.int64),
                                np.cumsum(ksched, axis=1)], axis=1)[:, :NBANK]
    ktot = int(kt.sum())

    # per-core slot tables
    idx_all, nrm_all, dinvt_all, spool_all = [], [], [], []
    cnt = np.maximum(np.bincount(batch, minlength=N_GRAPHS), 1).astype(np.float32)
    for c in range(NC):
        lo, hi, m, kb, pos = percore[c]
        es, ed = a_src[m], a_dst[m]
        epos = pos[ed - lo]
        et, ep = epos // P, epos % P
        eb = s_bank[m]
        # within-group slot index k
        order = np.lexsort((es, ep, eb, et))
        et_o, ep_o, eb_o, es_o = et[order], ep[order], eb[order], es[order]
        grp = (et_o * NBANK + eb_o) * P + ep_o
        first = np.r_[True, grp[1:] != grp[:-1]]
        starts = np.where(first, np.arange(grp.size), 0)
        k = np.arange(grp.size) - np.maximum.accumulate(starts)
        col = tile_base[et_o] + bank_base[et_o, eb_o] + k
        assert (k < ksched[et_o, eb_o]).all()

        nrm = np.zeros((P, ktot), np.float32)
        nrm[ep_o, col] = dinv[es_o]
        idx16 = np.zeros((16, 8 * ktot), np.int16)
        # call layout: for call (t,b), position i = k*128 + p at wrapped
        # (i%16, 8*colbase + i//16); colbase = tile_base+bank_base
        i_in = k * P + ep_o
        wcol = 8 * (tile_base[et_o] + bank_base[et_o, eb_o]) + i_in // 16
        idx16[i_in % 16, wcol] = (pid[es_o] - eb_o * BANK).astype(np.int16)
        idx_all.append(np.tile(idx16, (8, 1)))
        nrm_all.append(nrm)

        dinvt = np.zeros((P, TILES), np.float32)
        nloc = hi - lo
        dinvt[pos % P, pos // P] = dinv[lo:hi]
        dinvt_all.append(dinvt)

        spool = np.zeros((TILES * P, GPC), np.float32)
        gl = batch[lo:hi] - c * GPC
        spool[pos, gl] = 1.0 / cnt[batch[lo:hi]]
        spool_all.append(spool)

    # global permuted feature table
    x_g = np.zeros((TBL, D), np.float32)
    x_g[pid] = x
    return dict(
        x_g=x_g, idx=idx_all, nrm=nrm_all, dinvt=dinvt_all, spool=spool_all,
        ksched=ksched, tile_base=tile_base, bank_base=bank_base, ktot=ktot,
    )


def _build(ksched, tile_base, bank_base, ktot, zero_b1, zero_b2):
    import concourse.bacc as bacc
    import concourse.mybir as mybir
    import concourse.tile as tile
    from concourse import library_config
    from concourse.masks import make_identity
    from concourse.tile_rust import add_dep_helper

    f32 = mybir.dt.float32
    Act = mybir.ActivationFunctionType
    Alu = mybir.AluOpType

    nc = bacc.Bacc("TRN2", target_bir_lowering=False, debug=False,
                   num_devices=NC, num_swdge_queues=4)

    x_g = nc.dram_tensor("x_g", [TBL, D], f32, kind="ExternalInput")
    idx_in = nc.dram_tensor("idx", [P, 8 * ktot], mybir.dt.int16, kind="ExternalInput")
    nrm_in = nc.dram_tensor("nrm", [P, ktot], f32, kind="ExternalInput")
    dinvt_in = nc.dram_tensor("dinvt", [P, TILES], f32, kind="ExternalInput")
    spool_in = nc.dram_tensor("spool", [TILES * P, GPC], f32, kind="ExternalInput")
    w1_in = nc.dram_tensor("w1", [D, D], f32, kind="ExternalInput")
    w2_in = nc.dram_tensor("w2", [D, D], f32, kind="ExternalInput")
    wih_in = nc.dram_tensor("wih", [D, 3 * D], f32, kind="ExternalInput")  # [h, 3*fo] (W_ih.T blocks)
    bias_rz_in = nc.dram_tensor("bias_rz", [P, 2], f32, kind="ExternalInput")  # b_ih+b_hh for r,z
    bias_n_in = nc.dram_tensor("bias_n", [P, 2], f32, kind="ExternalInput")  # b_ih_n, b_hh_n
    wlin_in = nc.dram_tensor("wlin", [D, 1], f32, kind="ExternalInput")
    blin_in = nc.dram_tensor("blin", [1, 1], f32, kind="ExternalInput")
    b1_in = nc.dram_tensor("b1b", [P, D], f32, kind="ExternalInput")
    b2_in = nc.dram_tensor("b2b", [P, D], f32, kind="ExternalInput")
    out = nc.dram_tensor("out", [1, GPC], f32, kind="ExternalOutput")

    ag_in = nc.dram_tensor("ag_in", [NPC, D], f32, kind="Internal")
    h1g = nc.dram_tensor("h1g", [TBL, D], f32, kind="Internal", addr_space="Shared")

    nc.gpsimd.load_library(library_config.mlp)

    kmax = int(ksched.sum(axis=1).max())

    with tile.TileContext(nc) as tc:
        with (
            tc.tile_pool(name="io", bufs=1) as io,
            tc.tile_pool(name="gp", bufs=2) as gp,
            tc.tile_pool(name="wk", bufs=3) as wk,
            tc.tile_pool(name="sp", bufs=2) as sp,
            tc.tile_pool(name="ps_t", bufs=2, space="PSUM") as ps_t,
            tc.tile_pool(name="ps_m", bufs=2, space="PSUM") as ps_m,
            tc.tile_pool(name="ps_pool", bufs=1, space="PSUM") as ps_pool,
            tc.tile_pool(name="ps_h", bufs=2, space="PSUM") as ps_h,
        ):
            idx_t = io.tile([P, 8 * ktot], mybir.dt.int16)
            nrm_t = io.tile([P, ktot], f32)
            dinv_t = io.tile([P, TILES], f32)
            w1_t = io.tile([D, D], f32)
            w2_t = io.tile([D, D], f32)
            ident = io.tile([P, P], f32)
            nc.sync.dma_start(out=idx_t[:], in_=idx_in[:])
            nc.sync.dma_start(out=nrm_t[:], in_=nrm_in[:])
            nc.sync.dma_start(out=dinv_t[:], in_=dinvt_in[:])
            nc.sync.dma_start(out=w1_t[:], in_=w1_in[:])
            nc.sync.dma_start(out=w2_t[:], in_=w2_in[:])
            b1_t = io.tile([P, D], f32)
            b2_t = io.tile([P, D], f32)
            if not zero_b1:
                nc.sync.dma_start(out=b1_t[:], in_=b1_in[:])
            if not zero_b2:
                nc.sync.dma_start(out=b2_t[:], in_=b2_in[:])
            make_identity(nc, ident[:])

            call_ctr = [0]
            prev_gather = [None]

            def layer(table_rows, w_t, is_l1):
                pool_ps = None
                if not is_l1:
                    pool_ps = ps_pool.tile([P, GPC], f32, space="PSUM")
                for t in range(TILES):
                    ktt = int(ksched[t].sum())
                    g = gp.tile([P, kmax * D], f32, tag="g")
                    for b in range(NBANK):
                        kb = int(ksched[t, b])
                        cb = int(bank_base[t, b])
                        win = table_rows[b * BANK:(b + 1) * BANK, :]
                        # split calls to <= MAX_IDX_PER_CALL indices
                        kk0 = 0
                        while kk0 < kb:
                            kk = min(kb - kk0, MAX_IDX_PER_CALL // P)
                            c0 = cb + kk0
                            mz = nc.gpsimd.memzero(g[:, c0 * D:c0 * D + 1])
                            if prev_gather[0] is not None:
                                add_dep_helper(prev_gather[0].ins, mz.ins, False,
                                               "swdge spacing")
                            wbase = 8 * (int(tile_base[t]) + c0)
                            gi = nc.gpsimd.dma_gather(
                                out_ap=g[:, c0 * D:(c0 + kk) * D].rearrange(
                                    "p (k f) -> p k f", k=kk),
                                in_ap=win,
                                idxs_ap=idx_t[:, wbase:wbase + 8 * kk],
                                num_idxs=kk * P,
                                num_idxs_reg=kk * P,
                                elem_size=D,
                                queue_num=call_ctr[0] % 4,
                            )
                            prev_gather[0] = gi
                            call_ctr[0] += 1
                            kk0 += kk
                    # scale by dinv[src] (norm stream; pads are 0)
                    g3 = g[:, :ktt * D].rearrange("p (k f) -> p k f", k=ktt)
                    nb = nrm_t[:, int(tile_base[t]):int(tile_base[t]) + ktt] \
                        .unsqueeze(2).to_broadcast([P, ktt, D])
                    nc.vector.tensor_tensor(out=g3, in0=g3, in1=nb, op=Alu.mult)
                    # segment reduce over k
                    z = wk.tile([P, D], f32, tag="z")
                    gr = g[:, :ktt * D].rearrange("p (k f) -> p f k", k=ktt)
                    nc.vector.tensor_reduce(out=z[:], in_=gr,
                                            axis=mybir.AxisListType.X, op=Alu.add)
                    # W matmul: transpose z then lhsT
                    zt_ps = ps_t.tile([P, P], f32, space="PSUM", tag="zt")
                    nc.tensor.transpose(out=zt_ps[:], in_=z[:], identity=ident[:])
                    zt = wk.tile([P, P], f32, tag="zts")
                    nc.scalar.copy(out=zt[:], in_=zt_ps[:])
                    h_ps = ps_m.tile([P, D], f32, space="PSUM", tag="h")
                    nc.tensor.matmul(out=h_ps[:], lhsT=zt[:], rhs=w_t[:],
                                     start=True, stop=True)
                    h = wk.tile([P, D], f32, tag="h")
                    sc = dinv_t[:, t:t + 1]
                    if is_l1:
                        if zero_b1:
                            nc.scalar.activation(out=h[:], in_=h_ps[:], func=Act.Relu,
                                                 scale=sc)
                        else:
                            nc.vector.tensor_scalar_mul(h[:], h_ps[:], sc)
                            nc.vector.tensor_tensor(out=h[:], in0=h[:], in1=b1_t[:],
                                                    op=Alu.add)
                            nc.scalar.activation(out=h[:], in_=h[:], func=Act.Relu)
                        nc.sync.dma_start(out=ag_in[t * P:(t + 1) * P, :], in_=h[:])
                    else:
                        if zero_b2:
                            nc.scalar.mul(out=h[:], in_=h_ps[:], mul=sc)
                        else:
                            nc.vector.tensor_scalar_mul(h[:], h_ps[:], sc)
                            nc.vector.tensor_tensor(out=h[:], in0=h[:], in1=b2_t[:],
                                                    op=Alu.add)
                        spt = sp.tile([P, GPC], f32, tag="sp")
                        nc.sync.dma_start(out=spt[:],
                                          in_=spool_in[t * P:(t + 1) * P, :])
                        nc.tensor.matmul(out=pool_ps[:], lhsT=h[:], rhs=spt[:],
                                         start=(t == 0), stop=(t == TILES - 1))
                return pool_ps

            layer(x_g[:], w1_t, True)
            nc.gpsimd.collective_compute(
                "AllGather", Alu.bypass, replica_groups=[list(range(NC))],
                ins=[ag_in[:]], outs=[h1g[:]],
            )
            pool_ps = layer(h1g[:], w2_t, False)

            # ---- head: gT = pooled mean [128 h, 256 g] ----
            wih_t = io.tile([D, 3 * D], f32)
            brz_t = io.tile([P, 2], f32)
            bn_t = io.tile([P, 2], f32)
            wlin_t = io.tile([D, 1], f32)
            blin_t = io.tile([1, 1], f32)
            ones_m = io.tile([P, 1], f32)   # 1/128 for mean matmuls
            eps_t = io.tile([1, 1], f32)
            ones_r = io.tile([1, P], f32)   # row of ones for bcast matmuls
            nc.sync.dma_start(out=wih_t[:], in_=wih_in[:])
            nc.sync.dma_start(out=brz_t[:], in_=bias_rz_in[:])
            nc.sync.dma_start(out=bn_t[:], in_=bias_n_in[:])
            nc.sync.dma_start(out=wlin_t[:], in_=wlin_in[:])
            nc.sync.dma_start(out=blin_t[:], in_=blin_in[:])
            nc.vector.memset(ones_m[:], 1.0 / P)
            nc.vector.memset(eps_t[:], EPS)
            nc.vector.memset(ones_r[:], 1.0)

            hd = wk.tile([P, GPC], f32, tag="hd")
            nc.scalar.copy(out=hd[:], in_=pool_ps[:])  # gT [h, 256]

            def gate_mm(sl):
                ps = ps_h.tile([P, GPC], f32, space="PSUM", tag="hps")
                nc.tensor.matmul(out=ps[:], lhsT=wih_t[:, sl * D:(sl + 1) * D],
                                 rhs=hd[:], start=True, stop=True)
                return ps

            r = wk.tile([P, GPC], f32, tag="r")
            nc.scalar.activation(out=r[:], in_=gate_mm(0)[:], func=Act.Sigmoid,
                                 bias=brz_t[:, 0:1], scale=1.0)
            zz = wk.tile([P, GPC], f32, tag="zz")
            nc.scalar.activation(out=zz[:], in_=gate_mm(1)[:], func=Act.Sigmoid,
                                 bias=brz_t[:, 1:2], scale=1.0)
            nps = gate_mm(2)
            rb = wk.tile([P, GPC], f32, tag="rb")
            nc.vector.tensor_scalar_mul(rb[:], r[:], bn_t[:, 1:2])
            t1 = wk.tile([P, GPC], f32, tag="t1")
            nc.vector.tensor_tensor(out=t1[:], in0=nps[:], in1=rb[:], op=Alu.add)
            n_t = wk.tile([P, GPC], f32, tag="nt")
            nc.scalar.activation(out=n_t[:], in_=t1[:], func=Act.Tanh,
                                 bias=bn_t[:, 0:1], scale=1.0)
            zn = wk.tile([P, GPC], f32, tag="zn")
            nc.vector.tensor_tensor(out=zn[:], in0=zz[:], in1=n_t[:], op=Alu.mult)
            hr = wk.tile([P, GPC], f32, tag="hr")
            nc.vector.tensor_tensor(out=hr[:], in0=n_t[:], in1=zn[:],
                                    op=Alu.subtract)
            nc.scalar.activation(out=hr[:], in_=hr[:], func=Act.Relu)

            mu_ps = ps_h.tile([1, GPC], f32, space="PSUM", tag="hps")
            nc.tensor.matmul(out=mu_ps[:], lhsT=ones_m[:], rhs=hr[:],
                             start=True, stop=True)
            mu = wk.tile([1, GPC], f32, tag="mu")
            nc.scalar.copy(out=mu[:], in_=mu_ps[:])
            mub_ps = ps_h.tile([P, GPC], f32, space="PSUM", tag="hps")
            nc.tensor.matmul(out=mub_ps[:], lhsT=ones_r[:], rhs=mu[:],
                             start=True, stop=True)
            dmu = wk.tile([P, GPC], f32, tag="dmu")
            nc.vector.tensor_tensor(out=dmu[:], in0=hr[:], in1=mub_ps[:],
                                    op=Alu.subtract)
            d2 = wk.tile([P, GPC], f32, tag="d2")
            nc.scalar.activation(out=d2[:], in_=dmu[:], func=Act.Square)
            var_ps = ps_h.tile([1, GPC], f32, space="PSUM", tag="hps")
            nc.tensor.matmul(out=var_ps[:], lhsT=ones_m[:], rhs=d2[:],
                             start=True, stop=True)
            std = wk.tile([1, GPC], f32, tag="std")
            nc.scalar.activation(out=std[:], in_=var_ps[:], func=Act.Sqrt,
                                 bias=eps_t[:, 0:1])
            rstd = wk.tile([1, GPC], f32, tag="rstd")
            nc.vector.reciprocal(rstd[:], std[:])
            rsb_ps = ps_h.tile([P, GPC], f32, space="PSUM", tag="hps")
            nc.tensor.matmul(out=rsb_ps[:], lhsT=ones_r[:], rhs=rstd[:],
                             start=True, stop=True)
            gn = wk.tile([P, GPC], f32, tag="gn")
            nc.vector.tensor_tensor(out=gn[:], in0=dmu[:], in1=rsb_ps[:],
                                    op=Alu.mult)
            o_ps = ps_h.tile([1, GPC], f32, space="PSUM", tag="hps")
            nc.tensor.matmul(out=o_ps[:], lhsT=wlin_t[:], rhs=gn[:],
                             start=True, stop=True)
            o_sb = wk.tile([1, GPC], f32, tag="o")
            nc.scalar.activation(out=o_sb[:], in_=o_ps[:], func=Act.Identity,
                                 bias=blin_t[:, 0:1], scale=1.0)
            nc.sync.dma_start(out=out[:], in_=o_sb[:])

    nc.compile()
    return nc


def kernel(**inputs):
    x = np.ascontiguousarray(np.asarray(inputs["x"], dtype=np.float32))
    ei = np.asarray(inputs["edge_index"]).astype(np.int64)
    batch = np.asarray(inputs["batch"]).astype(np.int64)
    W1 = np.asarray(inputs["W1"], np.float32)
    b1 = np.asarray(inputs["b1"], np.float32)
    W2 = np.asarray(inputs["W2"], np.float32)
    b2 = np.asarray(inputs["b2"], np.float32)
    W_ih = np.asarray(inputs["W_ih"], np.float32)
    W_hh = np.asarray(inputs["W_hh"], np.float32)
    b_ih = np.asarray(inputs["b_ih"], np.float32)
    b_hh = np.asarray(inputs["b_hh"], np.float32)
    W_lin = np.asarray(inputs["W_lin"], np.float32)
    b_lin = np.asarray(inputs["b_lin"], np.float32)
    del W_hh  # unused: h0 == 0 makes gh = b_hh

    prep = _prep(x, ei[0], ei[1], batch)

    zero_b1 = not np.any(b1)
    zero_b2 = not np.any(b2)
    key = (prep["ktot"], zero_b1, zero_b2)
    if key not in _CACHE:
        _CACHE[key] = _build(prep["ksched"], prep["tile_base"],
                             prep["bank_base"], prep["ktot"], zero_b1, zero_b2)
    nc = _CACHE[key]

    # gate weight blocks as lhsT: [h, fo] per gate (r, z, n)
    wih = np.concatenate([W_ih[i * D:(i + 1) * D, :].T for i in range(3)],
                         axis=1).astype(np.float32)  # [128, 384]
    bias_rz = np.stack([b_ih[0:D] + b_hh[0:D], b_ih[D:2 * D] + b_hh[D:2 * D]],
                       axis=1).astype(np.float32)  # [128, 2]
    bias_n = np.stack([b_ih[2 * D:], b_hh[2 * D:]], axis=1).astype(np.float32)
    b1b = np.tile(b1[None, :], (P, 1)).astype(np.float32)
    b2b = np.tile(b2[None, :], (P, 1)).astype(np.float32)

    in_maps = []
    for c in range(NC):
        in_maps.append({
            "x_g": prep["x_g"],
            "idx": prep["idx"][c],
            "nrm": prep["nrm"][c],
            "dinvt": prep["dinvt"][c],
            "spool": prep["spool"][c],
            "w1": W1, "w2": W2,
            "wih": wih, "bias_rz": bias_rz, "bias_n": bias_n,
            "wlin": W_lin.T.astype(np.float32).reshape(D, 1),
            "blin": b_lin.reshape(1, 1).astype(np.float32),
            "b1b": b1b, "b2b": b2b,
        })

    global _last_in_maps
    _last_in_maps = in_maps
    from concourse.bass_utils import run_bass_kernel_spmd
    res = run_bass_kernel_spmd(nc, in_maps, core_ids=list(range(NC)))
    out = np.concatenate([res.results[c]["out"][0] for c in range(NC)])
    return out.reshape(N_GRAPHS, 1).astype(np.float32)


# revision 11
# speedup vs baseline: 1.0656x; 1.0656x over previous
"""Trainium2 Bass kernel for 2-layer GCN + mean-pool + GRU step + layernorm + linear.

Strategy (8 NeuronCores, SPMD single program, per-core data):
- Graph-aligned node sharding: core c owns graphs [256c, 256(c+1)) and their
  (contiguous, batch is sorted) node range. Per-core nodes padded to 12544.
- Both GCN layers commute the weight matmul past aggregation:
      h = relu(Agg(x) @ W + b),  Agg = D^-1/2 (A+I) D^-1/2
  so the gather reads raw features; self-loops are regular edges; the
  symmetric norm is dinv[src] (per-edge stream) times dinv[dst] (folded into
  the post-matmul activation scale).
- Gather: dma_gather (int16 idx, 4 table windows of 25088 rows = 2 core
  blocks each). Node-major K-slot layout: tile of 128 nodes gathers K
  neighbor rows per node; one DVE bcast-multiply (by dinv[src]) and one
  strided DVE reduce produce the aggregate.
- One AllGather of h1 between layers. Pooling via per-tile matmuls against a
  host-built [node,graph] selector with 1/cnt folded in. Head (GRU+LN+linear)
  computed per-core on its 256 graphs in feature-major layout.
"""

import numpy as np

N_NODES = 100000
N_EDGES = 1600000
N_GRAPHS = 2048
D = 128
P = 128
EPS = 1e-5
NC = 8
GPC = N_GRAPHS // NC          # graphs per core = 256
NPC = 13056                   # padded nodes per core (102 * 128)
TILES = NPC // P              # 102
TBL = NC * NPC                # global padded table rows
NBANK = 4
BANK = TBL // NBANK           # rows per dma_gather window (< 32768)
import os as _os
MAX_IDX_PER_CALL = int(_os.environ.get("MAXIDX", "1024"))

_CACHE = {}
_last_in_maps = None


def _prep(x, src, dst, batch):
    """Host-side graph-structure preprocessing (indices/degrees only)."""
    deg = np.bincount(dst, minlength=N_NODES).astype(np.float64) + 1.0
    dinv = (1.0 / np.sqrt(deg)).astype(np.float32)

    # graph-aligned shard boundaries
    node_start = np.searchsorted(batch, np.arange(0, N_GRAPHS + 1, GPC))
    core_of = np.searchsorted(node_start, np.arange(N_NODES), side="right") - 1

    # per-edge core/bank info (self-loops appended: one per node)
    a_src = np.concatenate([src, np.arange(N_NODES)])
    a_dst = np.concatenate([dst, np.arange(N_NODES)])
    e_core = core_of[a_dst]
    s_bank = core_of[a_src] // 2  # bank = 2 consecutive core blocks

    # per (core, local node) bank counts -> per-core perm (profile sort)
    perm_pos = np.zeros(N_NODES, np.int64)
    percore = []
    for c in range(NC):
        lo, hi = node_start[c], node_start[c + 1]
        nloc = hi - lo
        assert nloc <= NPC, f"core {c} has {nloc} nodes > NPC={NPC}"
        m = (a_dst >= lo) & (a_dst < hi)
        ldst = a_dst[m] - lo
        lbank = s_bank[m]
        kb = np.zeros((nloc, NBANK), np.int64)
        np.add.at(kb, (ldst, lbank), 1)
        order = np.lexsort((-kb[:, 3], -kb[:, 2], -kb[:, 1], -kb[:, 0]))
        pos = np.empty(nloc, np.int64)
        pos[order] = np.arange(nloc)
        perm_pos[lo:hi] = pos
        percore.append((lo, hi, m, kb, pos))

    pid = core_of * NPC + perm_pos  # global permuted id of each orig node

    # shared schedule: K_b per (tile, bank) = max over cores of per-core max count
    ksched = np.zeros((TILES, NBANK), np.int64)
    for c in range(NC):
        lo, hi, m, kb, pos = percore[c]
        nloc = hi - lo
        tile_of = pos // P
        for b in range(NBANK):
            kmax = np.zeros(TILES, np.int64)
            np.maximum.at(kmax, tile_of, kb[:, b])
            ksched[:, b] = np.maximum(ksched[:, b], kmax)
    ksched = np.maximum(ksched, 1)
    kt = ksched.sum(axis=1)                       # total slots per tile
    tile_base = np.concatenate([[0], np.cumsum(kt)])[:-1]
    bank_base = np.concatenate([np.zeros((TILES, 1), np.int64),
                                np.cumsum(ksched, axis=1)], axis=1)[:, :NBANK]
    ktot = int(kt.sum())

    # per-core slot tables
    idx_all, nrm_all, dinvt_all, spool_all = [], [], [], []
    cnt = np.maximum(np.bincount(batch, minlength=N_GRAPHS), 1).astype(np.float32)
    for c in range(NC):
        lo, hi, m, kb, pos = percore[c]
        es, ed = a_src[m], a_dst[m]
        epos = pos[ed - lo]
        et, ep = epos // P, epos % P
        eb = s_bank[m]
        # within-group slot index k
        order = np.lexsort((es, ep, eb, et))
        et_o, ep_o, eb_o, es_o = et[order], ep[order], eb[order], es[order]
        grp = (et_o * NBANK + eb_o) * P + ep_o
        first = np.r_[True, grp[1:] != grp[:-1]]
        starts = np.where(first, np.arange(grp.size), 0)
        k = np.arange(grp.size) - np.maximum.accumulate(starts)
        col = tile_base[et_o] + bank_base[et_o, eb_o] + k
        assert (k < ksched[et_o, eb_o]).all()

        nrm = np.zeros((P, ktot), np.float32)
        nrm[ep_o, col] = dinv[es_o]
        idx16 = np.zeros((16, 8 * ktot), np.int16)
        # call layout: for call (t,b), position i = k*128 + p at wrapped
        # (i%16, 8*colbase + i//16); colbase = tile_base+bank_base
        i_in = k * P + ep_o
        wcol = 8 * (tile_base[et_o] + bank_base[et_o, eb_o]) + i_in // 16
        idx16[i_in % 16, wcol] = (pid[es_o] - eb_o * BANK).astype(np.int16)
        idx_all.append(np.tile(idx16, (8, 1)))
        nrm_all.append(nrm)

        dinvt = np.zeros((P, TILES), np.float32)
        nloc = hi - lo
        dinvt[pos % P, pos // P] = dinv[lo:hi]
        dinvt_all.append(dinvt)

        spool = np.zeros((TILES * P, GPC), np.float32)
        gl = batch[lo:hi] - c * GPC
        spool[pos, gl] = 1.0 / cnt[batch[lo:hi]]
        spool_all.append(spool)

    # global permuted feature table
    x_g = np.zeros((TBL, D), np.float32)
    x_g[pid] = x
    return dict(
        x_g=x_g, idx=idx_all, nrm=nrm_all, dinvt=dinvt_all, spool=spool_all,
        ksched=ksched, tile_base=tile_base, bank_base=bank_base, ktot=ktot,
    )


def _build(ksched, tile_base, bank_base, ktot, zero_b1, zero_b2, variant="full"):
    import concourse.bacc as bacc
    import concourse.mybir as mybir
    import concourse.tile as tile
    from concourse import library_config
    from concourse.masks import make_identity
    from concourse.tile_rust import add_dep_helper

    f32 = mybir.dt.float32
    Act = mybir.ActivationFunctionType
    Alu = mybir.AluOpType

    nc = bacc.Bacc("TRN2", target_bir_lowering=False, debug=False,
                   num_devices=NC, num_swdge_queues=4)

    x_g = nc.dram_tensor("x_g", [TBL, D], f32, kind="ExternalInput")
    idx_in = nc.dram_tensor("idx", [P, 8 * ktot], mybir.dt.int16, kind="ExternalInput")
    nrm_in = nc.dram_tensor("nrm", [P, ktot], f32, kind="ExternalInput")
    dinvt_in = nc.dram_tensor("dinvt", [P, TILES], f32, kind="ExternalInput")
    spool_in = nc.dram_tensor("spool", [TILES * P, GPC], f32, kind="ExternalInput")
    w1_in = nc.dram_tensor("w1", [D, D], f32, kind="ExternalInput")
    w2_in = nc.dram_tensor("w2", [D, D], f32, kind="ExternalInput")
    wih_in = nc.dram_tensor("wih", [D, 3 * D], f32, kind="ExternalInput")  # [h, 3*fo] (W_ih.T blocks)
    bias_rz_in = nc.dram_tensor("bias_rz", [P, 2], f32, kind="ExternalInput")  # b_ih+b_hh for r,z
    bias_n_in = nc.dram_tensor("bias_n", [P, 2], f32, kind="ExternalInput")  # b_ih_n, b_hh_n
    wlin_in = nc.dram_tensor("wlin", [D, 1], f32, kind="ExternalInput")
    blin_in = nc.dram_tensor("blin", [1, 1], f32, kind="ExternalInput")
    b1_in = nc.dram_tensor("b1b", [P, D], f32, kind="ExternalInput")
    b2_in = nc.dram_tensor("b2b", [P, D], f32, kind="ExternalInput")
    out = nc.dram_tensor("out", [1, GPC], f32, kind="ExternalOutput")

    ag_in = nc.dram_tensor("ag_in", [NPC, D], f32, kind="Internal")
    h1g = nc.dram_tensor("h1g", [TBL, D], f32, kind="Internal", addr_space="Shared")

    nc.gpsimd.load_library(library_config.mlp)

    kmax = int(ksched.sum(axis=1).max())

    with tile.TileContext(nc) as tc:
        with (
            tc.tile_pool(name="io", bufs=1) as io,
            tc.tile_pool(name="gp", bufs=int(_os.environ.get("GBUFS", "2"))) as gp,
            tc.tile_pool(name="wk", bufs=3) as wk,
            tc.tile_pool(name="sp", bufs=2) as sp,
            tc.tile_pool(name="ps_t", bufs=2, space="PSUM") as ps_t,
            tc.tile_pool(name="ps_m", bufs=2, space="PSUM") as ps_m,
            tc.tile_pool(name="ps_pool", bufs=1, space="PSUM") as ps_pool,
            tc.tile_pool(name="ps_h", bufs=2, space="PSUM") as ps_h,
        ):
            idx_t = io.tile([P, 8 * ktot], mybir.dt.int16)
            nrm_t = io.tile([P, ktot], f32)
            dinv_t = io.tile([P, TILES], f32)
            w1_t = io.tile([D, D], f32)
            w2_t = io.tile([D, D], f32)
            ident = io.tile([P, P], f32)
            nc.sync.dma_start(out=idx_t[:], in_=idx_in[:])
            nc.sync.dma_start(out=nrm_t[:], in_=nrm_in[:])
            nc.sync.dma_start(out=dinv_t[:], in_=dinvt_in[:])
            nc.sync.dma_start(out=w1_t[:], in_=w1_in[:])
            nc.sync.dma_start(out=w2_t[:], in_=w2_in[:])
            b1_t = io.tile([P, D], f32)
            b2_t = io.tile([P, D], f32)
            if not zero_b1:
                nc.sync.dma_start(out=b1_t[:], in_=b1_in[:])
            if not zero_b2:
                nc.sync.dma_start(out=b2_t[:], in_=b2_in[:])
            make_identity(nc, ident[:])

            call_ctr = [0]
            prev_gather = [None]

            def layer(table_rows, w_t, is_l1, variant=variant):
                pool_ps = None
                if not is_l1:
                    pool_ps = ps_pool.tile([P, GPC], f32, space="PSUM")
                for t in range(TILES):
                    ktt = int(ksched[t].sum())
                    g = gp.tile([P, kmax * D], f32, tag="g")
                    for b in range(NBANK):
                        kb = int(ksched[t, b])
                        cb = int(bank_base[t, b])
                        win = table_rows[b * BANK:(b + 1) * BANK, :]
                        # split calls to <= MAX_IDX_PER_CALL indices
                        kk0 = 0
                        while kk0 < kb:
                            kk = min(kb - kk0, MAX_IDX_PER_CALL // P)
                            c0 = cb + kk0
                            mz = nc.gpsimd.memzero(
                                g[0:1, c0 * D:c0 * D + 1]
                                if variant != "bigmz" else g[:, c0 * D:c0 * D + 1])
                            if prev_gather[0] is not None:
                                add_dep_helper(prev_gather[0].ins, mz.ins, False,
                                               "swdge spacing")
                            wbase = 8 * (int(tile_base[t]) + c0)
                            gi = nc.gpsimd.dma_gather(
                                out_ap=g[:, c0 * D:(c0 + kk) * D].rearrange(
                                    "p (k f) -> p k f", k=kk),
                                in_ap=win,
                                idxs_ap=idx_t[:, wbase:wbase + 8 * kk],
                                num_idxs=kk * P,
                                num_idxs_reg=kk * P,
                                elem_size=D,
                                queue_num=(t % 4) if variant == "qtile" else (0 if variant == "q0" else call_ctr[0] % 4),
                            )
                            prev_gather[0] = gi
                            call_ctr[0] += 1
                            kk0 += kk
                    # scale by dinv[src] (norm stream; pads are 0)
                    z = wk.tile([P, D], f32, tag="z")
                    if variant == "nodve":
                        nc.vector.tensor_copy(z[:], g[:, :D])
                    else:
                        g3 = g[:, :ktt * D].rearrange("p (k f) -> p k f", k=ktt)
                        nb = nrm_t[:, int(tile_base[t]):int(tile_base[t]) + ktt] \
                            .unsqueeze(2).to_broadcast([P, ktt, D])
                        nc.vector.tensor_tensor(out=g3, in0=g3, in1=nb, op=Alu.mult)
                        gr = g[:, :ktt * D].rearrange("p (k f) -> p f k", k=ktt)
                        nc.vector.tensor_reduce(out=z[:], in_=gr,
                                                axis=mybir.AxisListType.X, op=Alu.add)
                    # W matmul: transpose z then lhsT
                    zt_ps = ps_t.tile([P, P], f32, space="PSUM", tag="zt")
                    nc.tensor.transpose(out=zt_ps[:], in_=z[:], identity=ident[:])
                    zt = wk.tile([P, P], f32, tag="zts")
                    nc.scalar.copy(out=zt[:], in_=zt_ps[:])
                    h_ps = ps_m.tile([P, D], f32, space="PSUM", tag="h")
                    nc.tensor.matmul(out=h_ps[:], lhsT=zt[:], rhs=w_t[:],
                                     start=True, stop=True)
                    h = wk.tile([P, D], f32, tag="h")
                    sc = dinv_t[:, t:t + 1]
                    if is_l1:
                        if zero_b1:
                            nc.scalar.activation(out=h[:], in_=h_ps[:], func=Act.Relu,
                                                 scale=sc)
                        else:
                            nc.vector.tensor_scalar_mul(h[:], h_ps[:], sc)
                            nc.vector.tensor_tensor(out=h[:], in0=h[:], in1=b1_t[:],
                                                    op=Alu.add)
                            nc.scalar.activation(out=h[:], in_=h[:], func=Act.Relu)
                        nc.sync.dma_start(out=ag_in[t * P:(t + 1) * P, :], in_=h[:])
                    else:
                        if zero_b2:
                            nc.scalar.mul(out=h[:], in_=h_ps[:], mul=sc)
                        else:
                            nc.vector.tensor_scalar_mul(h[:], h_ps[:], sc)
                            nc.vector.tensor_tensor(out=h[:], in0=h[:], in1=b2_t[:],
                                                    op=Alu.add)
                        spt = sp.tile([P, GPC], f32, tag="sp")
                        nc.sync.dma_start(out=spt[:],
                                          in_=spool_in[t * P:(t + 1) * P, :])
                        nc.tensor.matmul(out=pool_ps[:], lhsT=h[:], rhs=spt[:],
                                         start=(t == 0), stop=(t == TILES - 1))
                return pool_ps

            layer(x_g[:], w1_t, True)
            if variant == "noag":
                for c8 in range(NC):
                    nc.sync.dma_start(out=h1g[c8 * NPC:(c8 + 1) * NPC, :],
                                      in_=ag_in[:])
            else:
                nc.gpsimd.collective_compute(
                    "AllGather", Alu.bypass, replica_groups=[list(range(NC))],
                    ins=[ag_in[:]], outs=[h1g[:]],
                )
            pool_ps = layer(h1g[:], w2_t, False)

            # ---- head: gT = pooled mean [128 h, 256 g] ----
            wih_t = io.tile([D, 3 * D], f32)
            brz_t = io.tile([P, 2], f32)
            bn_t = io.tile([P, 2], f32)
            wlin_t = io.tile([D, 1], f32)
            blin_t = io.tile([1, 1], f32)
            ones_m = io.tile([P, 1], f32)   # 1/128 for mean matmuls
            eps_t = io.tile([1, 1], f32)
            ones_r = io.tile([1, P], f32)   # row of ones for bcast matmuls
            nc.sync.dma_start(out=wih_t[:], in_=wih_in[:])
            nc.sync.dma_start(out=brz_t[:], in_=bias_rz_in[:])
            nc.sync.dma_start(out=bn_t[:], in_=bias_n_in[:])
            nc.sync.dma_start(out=wlin_t[:], in_=wlin_in[:])
            nc.sync.dma_start(out=blin_t[:], in_=blin_in[:])
            nc.vector.memset(ones_m[:], 1.0 / P)
            nc.vector.memset(eps_t[:], EPS)
            nc.vector.memset(ones_r[:], 1.0)

            hd = wk.tile([P, GPC], f32, tag="hd")
            nc.scalar.copy(out=hd[:], in_=pool_ps[:])  # gT [h, 256]

            def gate_mm(sl):
                ps = ps_h.tile([P, GPC], f32, space="PSUM", tag="hps")
                nc.tensor.matmul(out=ps[:], lhsT=wih_t[:, sl * D:(sl + 1) * D],
                                 rhs=hd[:], start=True, stop=True)
                return ps

            r = wk.tile([P, GPC], f32, tag="r")
            nc.scalar.activation(out=r[:], in_=gate_mm(0)[:], func=Act.Sigmoid,
                                 bias=brz_t[:, 0:1], scale=1.0)
            zz = wk.tile([P, GPC], f32, tag="zz")
            nc.scalar.activation(out=zz[:], in_=gate_mm(1)[:], func=Act.Sigmoid,
                                 bias=brz_t[:, 1:2], scale=1.0)
            nps = gate_mm(2)
            rb = wk.tile([P, GPC], f32, tag="rb")
            nc.vector.tensor_scalar_mul(rb[:], r[:], bn_t[:, 1:2])
            t1 = wk.tile([P, GPC], f32, tag="t1")
            nc.vector.tensor_tensor(out=t1[:], in0=nps[:], in1=rb[:], op=Alu.add)
            n_t = wk.tile([P, GPC], f32, tag="nt")
            nc.scalar.activation(out=n_t[:], in_=t1[:], func=Act.Tanh,
                                 bias=bn_t[:, 0:1], scale=1.0)
            zn = wk.tile([P, GPC], f32, tag="zn")
            nc.vector.tensor_tensor(out=zn[:], in0=zz[:], in1=n_t[:], op=Alu.mult)
            hr = wk.tile([P, GPC], f32, tag="hr")
            nc.vector.tensor_tensor(out=hr[:], in0=n_t[:], in1=zn[:],
                                    op=Alu.subtract)
            nc.scalar.activation(out=hr[:], in_=hr[:], func=Act.Relu)

            mu_ps = ps_h.tile([1, GPC], f32, space="PSUM", tag="hps")
            nc.tensor.matmul(out=mu_ps[:], lhsT=ones_m[:], rhs=hr[:],
                             start=True, stop=True)
            mu = wk.tile([1, GPC], f32, tag="mu")
            nc.scalar.copy(out=mu[:], in_=mu_ps[:])
            mub_ps = ps_h.tile([P, GPC], f32, space="PSUM", tag="hps")
            nc.tensor.matmul(out=mub_ps[:], lhsT=ones_r[:], rhs=mu[:],
                             start=True, stop=True)
            dmu = wk.tile([P, GPC], f32, tag="dmu")
            nc.vector.tensor_tensor(out=dmu[:], in0=hr[:], in1=mub_ps[:],
                                    op=Alu.subtract)
            d2 = wk.tile([P, GPC], f32, tag="d2")
            nc.scalar.activation(out=d2[:], in_=dmu[:], func=Act.Square)
            var_ps = ps_h.tile([1, GPC], f32, space="PSUM", tag="hps")
            nc.tensor.matmul(out=var_ps[:], lhsT=ones_m[:], rhs=d2[:],
                             start=True, stop=True)
            std = wk.tile([1, GPC], f32, tag="std")
            nc.scalar.activation(out=std[:], in_=var_ps[:], func=Act.Sqrt,
                                 bias=eps_t[:, 0:1])
            rstd = wk.tile([1, GPC], f32, tag="rstd")
            nc.vector.reciprocal(rstd[:], std[:])
            rsb_ps = ps_h.tile([P, GPC], f32, space="PSUM", tag="hps")
            nc.tensor.matmul(out=rsb_ps[:], lhsT=ones_r[:], rhs=rstd[:],
                             start=True, stop=True)
            gn = wk.tile([P, GPC], f32, tag="gn")
            nc.vector.tensor_tensor(out=gn[:], in0=dmu[:], in1=rsb_ps[:],
                                    op=Alu.mult)
            o_ps = ps_h.tile([1, GPC], f32, space="PSUM", tag="hps")
            nc.tensor.matmul(out=o_ps[:], lhsT=wlin_t[:], rhs=gn[:],
                             start=True, stop=True)
            o_sb = wk.tile([1, GPC], f32, tag="o")
            nc.scalar.activation(out=o_sb[:], in_=o_ps[:], func=Act.Identity,
                                 bias=blin_t[:, 0:1], scale=1.0)
            nc.sync.dma_start(out=out[:], in_=o_sb[:])

    nc.compile()
    return nc


def kernel(**inputs):
    x = np.ascontiguousarray(np.asarray(inputs["x"], dtype=np.float32))
    ei = np.asarray(inputs["edge_index"]).astype(np.int64)
    batch = np.asarray(inputs["batch"]).astype(np.int64)
    W1 = np.asarray(inputs["W1"], np.float32)
    b1 = np.asarray(inputs["b1"], np.float32)
    W2 = np.asarray(inputs["W2"], np.float32)
    b2 = np.asarray(inputs["b2"], np.float32)
    W_ih = np.asarray(inputs["W_ih"], np.float32)
    W_hh = np.asarray(inputs["W_hh"], np.float32)
    b_ih = np.asarray(inputs["b_ih"], np.float32)
    b_hh = np.asarray(inputs["b_hh"], np.float32)
    W_lin = np.asarray(inputs["W_lin"], np.float32)
    b_lin = np.asarray(inputs["b_lin"], np.float32)
    del W_hh  # unused: h0 == 0 makes gh = b_hh

    prep = _prep(x, ei[0], ei[1], batch)

    zero_b1 = not np.any(b1)
    zero_b2 = not np.any(b2)
    key = (prep["ktot"], zero_b1, zero_b2)
    if key not in _CACHE:
        _CACHE[key] = _build(prep["ksched"], prep["tile_base"],
                             prep["bank_base"], prep["ktot"], zero_b1, zero_b2)
    nc = _CACHE[key]

    # gate weight blocks as lhsT: [h, fo] per gate (r, z, n)
    wih = np.concatenate([W_ih[i * D:(i + 1) * D, :].T for i in range(3)],
                         axis=1).astype(np.float32)  # [128, 384]
    bias_rz = np.stack([b_ih[0:D] + b_hh[0:D], b_ih[D:2 * D] + b_hh[D:2 * D]],
                       axis=1).astype(np.float32)  # [128, 2]
    bias_n = np.stack([b_ih[2 * D:], b_hh[2 * D:]], axis=1).astype(np.float32)
    b1b = np.tile(b1[None, :], (P, 1)).astype(np.float32)
    b2b = np.tile(b2[None, :], (P, 1)).astype(np.float32)

    in_maps = []
    for c in range(NC):
        in_maps.append({
            "x_g": prep["x_g"],
            "idx": prep["idx"][c],
            "nrm": prep["nrm"][c],
            "dinvt": prep["dinvt"][c],
            "spool": prep["spool"][c],
            "w1": W1, "w2": W2,
            "wih": wih, "bias_rz": bias_rz, "bias_n": bias_n,
            "wlin": W_lin.T.astype(np.float32).reshape(D, 1),
            "blin": b_lin.reshape(1, 1).astype(np.float32),
            "b1b": b1b, "b2b": b2b,
        })

    global _last_in_maps
    _last_in_maps = in_maps
    from concourse.bass_utils import run_bass_kernel_spmd
    res = run_bass_kernel_spmd(nc, in_maps, core_ids=list(range(NC)))
    out = np.concatenate([res.results[c]["out"][0] for c in range(NC)])
    return out.reshape(N_GRAPHS, 1).astype(np.float32)


# revision 13
# speedup vs baseline: 1.0979x; 1.0303x over previous
"""Trainium2 Bass kernel for 2-layer GCN + mean-pool + GRU step + layernorm + linear.

Strategy (8 NeuronCores, SPMD single program, per-core data):
- Graph-aligned node sharding: core c owns graphs [256c, 256(c+1)) and their
  (contiguous, batch is sorted) node range. Per-core nodes padded to 12544.
- Both GCN layers commute the weight matmul past aggregation:
      h = relu(Agg(x) @ W + b),  Agg = D^-1/2 (A+I) D^-1/2
  so the gather reads raw features; self-loops are regular edges; the
  symmetric norm is dinv[src] (per-edge stream) times dinv[dst] (folded into
  the post-matmul activation scale).
- Gather: dma_gather (int16 idx, 4 table windows of 25088 rows = 2 core
  blocks each). Node-major K-slot layout: tile of 128 nodes gathers K
  neighbor rows per node; one DVE bcast-multiply (by dinv[src]) and one
  strided DVE reduce produce the aggregate.
- One AllGather of h1 between layers. Pooling via per-tile matmuls against a
  host-built [node,graph] selector with 1/cnt folded in. Head (GRU+LN+linear)
  computed per-core on its 256 graphs in feature-major layout.
"""

import numpy as np

N_NODES = 100000
N_EDGES = 1600000
N_GRAPHS = 2048
D = 128
P = 128
EPS = 1e-5
NC = 8
GPC = N_GRAPHS // NC          # graphs per core = 256
NPC = 13056                   # padded nodes per core (102 * 128)
TILES = NPC // P              # 102
TBL = NC * NPC                # global padded table rows
NBANK = 4
BANK = TBL // NBANK           # rows per dma_gather window (< 32768)
import os as _os
MAX_IDX_PER_CALL = int(_os.environ.get("MAXIDX", "1024"))

_CACHE = {}
_last_in_maps = None


def _prep(x, src, dst, batch):
    """Host-side graph-structure preprocessing (indices/degrees only)."""
    deg = np.bincount(dst, minlength=N_NODES).astype(np.float64) + 1.0
    dinv = (1.0 / np.sqrt(deg)).astype(np.float32)

    # graph-aligned shard boundaries
    node_start = np.searchsorted(batch, np.arange(0, N_GRAPHS + 1, GPC))
    core_of = np.searchsorted(node_start, np.arange(N_NODES), side="right") - 1

    # per-edge core/bank info (self-loops appended: one per node)
    a_src = np.concatenate([src, np.arange(N_NODES)])
    a_dst = np.concatenate([dst, np.arange(N_NODES)])
    e_core = core_of[a_dst]
    s_bank = core_of[a_src] // 2  # bank = 2 consecutive core blocks

    # per (core, local node) bank counts -> per-core perm (profile sort)
    perm_pos = np.zeros(N_NODES, np.int64)
    percore = []
    for c in range(NC):
        lo, hi = node_start[c], node_start[c + 1]
        nloc = hi - lo
        assert nloc <= NPC, f"core {c} has {nloc} nodes > NPC={NPC}"
        m = (a_dst >= lo) & (a_dst < hi)
        ldst = a_dst[m] - lo
        lbank = s_bank[m]
        kb = np.zeros((nloc, NBANK), np.int64)
        np.add.at(kb, (ldst, lbank), 1)
        order = np.lexsort((-kb[:, 3], -kb[:, 2], -kb[:, 1], -kb[:, 0]))
        pos = np.empty(nloc, np.int64)
        pos[order] = np.arange(nloc)
        perm_pos[lo:hi] = pos
        percore.append((lo, hi, m, kb, pos))

    pid = core_of * NPC + perm_pos  # global permuted id of each orig node

    # shared schedule: K_b per (tile, bank) = max over cores of per-core max count
    ksched = np.zeros((TILES, NBANK), np.int64)
    for c in range(NC):
        lo, hi, m, kb, pos = percore[c]
        nloc = hi - lo
        tile_of = pos // P
        for b in range(NBANK):
            kmax = np.zeros(TILES, np.int64)
            np.maximum.at(kmax, tile_of, kb[:, b])
            ksched[:, b] = np.maximum(ksched[:, b], kmax)
    ksched = np.maximum(ksched, 1)
    kt = ksched.sum(axis=1)                       # total slots per tile
    tile_base = np.concatenate([[0], np.cumsum(kt)])[:-1]
    bank_base = np.concatenate([np.zeros((TILES, 1), np.int64),
                                np.cumsum(ksched, axis=1)], axis=1)[:, :NBANK]
    ktot = int(kt.sum())

    # per-core slot tables
    idx_all, nrm_all, dinvt_all, spool_all = [], [], [], []
    cnt = np.maximum(np.bincount(batch, minlength=N_GRAPHS), 1).astype(np.float32)
    for c in range(NC):
        lo, hi, m, kb, pos = percore[c]
        es, ed = a_src[m], a_dst[m]
        epos = pos[ed - lo]
        et, ep = epos // P, epos % P
        eb = s_bank[m]
        # within-group slot index k
        order = np.lexsort((es, ep, eb, et))
        et_o, ep_o, eb_o, es_o = et[order], ep[order], eb[order], es[order]
        grp = (et_o * NBANK + eb_o) * P + ep_o
        first = np.r_[True, grp[1:] != grp[:-1]]
        starts = np.where(first, np.arange(grp.size), 0)
        k = np.arange(grp.size) - np.maximum.accumulate(starts)
        col = tile_base[et_o] + bank_base[et_o, eb_o] + k
        assert (k < ksched[et_o, eb_o]).all()

        nrm = np.zeros((P, ktot), np.float32)
        nrm[ep_o, col] = dinv[es_o]
        idx16 = np.zeros((16, 8 * ktot), np.int16)
        # call layout: for call (t,b), position i = k*128 + p at wrapped
        # (i%16, 8*colbase + i//16); colbase = tile_base+bank_base
        i_in = k * P + ep_o
        wcol = 8 * (tile_base[et_o] + bank_base[et_o, eb_o]) + i_in // 16
        idx16[i_in % 16, wcol] = (pid[es_o] - eb_o * BANK).astype(np.int16)
        idx_all.append(np.tile(idx16, (8, 1)))
        nrm_all.append(nrm)

        dinvt = np.zeros((P, TILES), np.float32)
        nloc = hi - lo
        dinvt[pos % P, pos // P] = dinv[lo:hi]
        dinvt_all.append(dinvt)

        spool = np.zeros((TILES * P, GPC), np.float32)
        gl = batch[lo:hi] - c * GPC
        spool[pos, gl] = 1.0 / cnt[batch[lo:hi]]
        spool_all.append(spool)

    # global permuted feature table
    x_g = np.zeros((TBL, D), np.float32)
    x_g[pid] = x
    return dict(
        x_g=x_g, idx=idx_all, nrm=nrm_all, dinvt=dinvt_all, spool=spool_all,
        ksched=ksched, tile_base=tile_base, bank_base=bank_base, ktot=ktot,
    )


def _build(ksched, tile_base, bank_base, ktot, zero_b1, zero_b2, variant="full"):
    import concourse.bacc as bacc
    import concourse.mybir as mybir
    import concourse.tile as tile
    from concourse import library_config
    from concourse.masks import make_identity
    from concourse.tile_rust import add_dep_helper

    f32 = mybir.dt.float32
    Act = mybir.ActivationFunctionType
    Alu = mybir.AluOpType

    nc = bacc.Bacc("TRN2", target_bir_lowering=False, debug=False,
                   num_devices=NC, num_swdge_queues=4)

    x_g = nc.dram_tensor("x_g", [TBL, D], f32, kind="ExternalInput")
    idx_in = nc.dram_tensor("idx", [P, 8 * ktot], mybir.dt.int16, kind="ExternalInput")
    nrm_in = nc.dram_tensor("nrm", [P, ktot], f32, kind="ExternalInput")
    dinvt_in = nc.dram_tensor("dinvt", [P, TILES], f32, kind="ExternalInput")
    spool_in = nc.dram_tensor("spool", [TILES * P, GPC], f32, kind="ExternalInput")
    w1_in = nc.dram_tensor("w1", [D, D], f32, kind="ExternalInput")
    w2_in = nc.dram_tensor("w2", [D, D], f32, kind="ExternalInput")
    wih_in = nc.dram_tensor("wih", [D, 3 * D], f32, kind="ExternalInput")  # [h, 3*fo] (W_ih.T blocks)
    bias_rz_in = nc.dram_tensor("bias_rz", [P, 2], f32, kind="ExternalInput")  # b_ih+b_hh for r,z
    bias_n_in = nc.dram_tensor("bias_n", [P, 2], f32, kind="ExternalInput")  # b_ih_n, b_hh_n
    wlin_in = nc.dram_tensor("wlin", [D, 1], f32, kind="ExternalInput")
    blin_in = nc.dram_tensor("blin", [1, 1], f32, kind="ExternalInput")
    b1_in = nc.dram_tensor("b1b", [P, D], f32, kind="ExternalInput")
    b2_in = nc.dram_tensor("b2b", [P, D], f32, kind="ExternalInput")
    out = nc.dram_tensor("out", [1, GPC], f32, kind="ExternalOutput")

    ag_in = nc.dram_tensor("ag_in", [NPC, D], f32, kind="Internal")
    h1g = nc.dram_tensor("h1g", [TBL, D], f32, kind="Internal", addr_space="Shared")

    nc.gpsimd.load_library(library_config.mlp)

    kmax = int(ksched.sum(axis=1).max())

    with tile.TileContext(nc) as tc:
        with (
            tc.tile_pool(name="io", bufs=1) as io,
            tc.tile_pool(name="gp", bufs=int(_os.environ.get("GBUFS", "2"))) as gp,
            tc.tile_pool(name="wk", bufs=3) as wk,
            tc.tile_pool(name="sp", bufs=2) as sp,
            tc.tile_pool(name="ps_t", bufs=2, space="PSUM") as ps_t,
            tc.tile_pool(name="ps_m", bufs=2, space="PSUM") as ps_m,
            tc.tile_pool(name="ps_pool", bufs=1, space="PSUM") as ps_pool,
            tc.tile_pool(name="ps_h", bufs=2, space="PSUM") as ps_h,
        ):
            idx_t = io.tile([P, 8 * ktot], mybir.dt.int16)
            nrm_t = io.tile([P, ktot], f32)
            dinv_t = io.tile([P, TILES], f32)
            w1_t = io.tile([D, D], f32)
            w2_t = io.tile([D, D], f32)
            ident = io.tile([P, P], f32)
            nc.sync.dma_start(out=idx_t[:], in_=idx_in[:])
            nc.sync.dma_start(out=nrm_t[:], in_=nrm_in[:])
            nc.sync.dma_start(out=dinv_t[:], in_=dinvt_in[:])
            nc.sync.dma_start(out=w1_t[:], in_=w1_in[:])
            nc.sync.dma_start(out=w2_t[:], in_=w2_in[:])
            b1_t = io.tile([P, D], f32)
            b2_t = io.tile([P, D], f32)
            if not zero_b1:
                nc.sync.dma_start(out=b1_t[:], in_=b1_in[:])
            if not zero_b2:
                nc.sync.dma_start(out=b2_t[:], in_=b2_in[:])
            make_identity(nc, ident[:])

            call_ctr = [0]
            prev_gather = [None]

            def layer(table_rows, w_t, is_l1, variant=variant):
                pool_ps = None
                if not is_l1:
                    pool_ps = ps_pool.tile([P, GPC], f32, space="PSUM")
                for t in range(TILES):
                    ktt = int(ksched[t].sum())
                    g = gp.tile([P, kmax * D], f32, tag="g")
                    z = wk.tile([P, D], f32, tag="z")
                    for b in range(NBANK):
                        kb = int(ksched[t, b])
                        cb = int(bank_base[t, b])
                        win = table_rows[b * BANK:(b + 1) * BANK, :]
                        # split calls to <= MAX_IDX_PER_CALL indices
                        kk0 = 0
                        while kk0 < kb:
                            kk = min(kb - kk0, MAX_IDX_PER_CALL // P)
                            c0 = cb + kk0
                            mz = nc.gpsimd.memzero(
                                g[0:1, c0 * D:c0 * D + 1]
                                if variant != "bigmz" else g[:, c0 * D:c0 * D + 1])
                            if prev_gather[0] is not None:
                                add_dep_helper(prev_gather[0].ins, mz.ins, False,
                                               "swdge spacing")
                            wbase = 8 * (int(tile_base[t]) + c0)
                            gi = nc.gpsimd.dma_gather(
                                out_ap=g[:, c0 * D:(c0 + kk) * D].rearrange(
                                    "p (k f) -> p k f", k=kk),
                                in_ap=win,
                                idxs_ap=idx_t[:, wbase:wbase + 8 * kk],
                                num_idxs=kk * P,
                                num_idxs_reg=kk * P,
                                elem_size=D,
                                queue_num=(t % 4) if variant == "qtile" else (0 if variant == "q0" else call_ctr[0] % 4),
                            )
                            prev_gather[0] = gi
                            call_ctr[0] += 1
                            kk0 += kk
                        # per-bank partial mul+reduce overlaps later banks' gathers
                        if variant == "nodve":
                            continue
                        gseg = g[:, cb * D:(cb + kb) * D]
                        g3 = gseg.rearrange("p (k f) -> p k f", k=kb)
                        base = int(tile_base[t]) + cb
                        nb = nrm_t[:, base:base + kb] \
                            .unsqueeze(2).to_broadcast([P, kb, D])
                        nc.vector.tensor_tensor(out=g3, in0=g3, in1=nb, op=Alu.mult)
                        gr = gseg.rearrange("p (k f) -> p f k", k=kb)
                        if b == 0:
                            nc.vector.tensor_reduce(out=z[:], in_=gr,
                                                    axis=mybir.AxisListType.X,
                                                    op=Alu.add)
                        else:
                            zb = wk.tile([P, D], f32, tag="zb")
                            nc.vector.tensor_reduce(out=zb[:], in_=gr,
                                                    axis=mybir.AxisListType.X,
                                                    op=Alu.add)
                            nc.vector.tensor_tensor(out=z[:], in0=z[:], in1=zb[:],
                                                    op=Alu.add)
                    if variant == "nodve":
                        nc.vector.tensor_copy(z[:], g[:, :D])
                    # W matmul: transpose z then lhsT
                    zt_ps = ps_t.tile([P, P], f32, space="PSUM", tag="zt")
                    nc.tensor.transpose(out=zt_ps[:], in_=z[:], identity=ident[:])
                    zt = wk.tile([P, P], f32, tag="zts")
                    nc.scalar.copy(out=zt[:], in_=zt_ps[:])
                    h_ps = ps_m.tile([P, D], f32, space="PSUM", tag="h")
                    nc.tensor.matmul(out=h_ps[:], lhsT=zt[:], rhs=w_t[:],
                                     start=True, stop=True)
                    h = wk.tile([P, D], f32, tag="h")
                    sc = dinv_t[:, t:t + 1]
                    if is_l1:
                        if zero_b1:
                            nc.scalar.activation(out=h[:], in_=h_ps[:], func=Act.Relu,
                                                 scale=sc)
                        else:
                            nc.vector.tensor_scalar_mul(h[:], h_ps[:], sc)
                            nc.vector.tensor_tensor(out=h[:], in0=h[:], in1=b1_t[:],
                                                    op=Alu.add)
                            nc.scalar.activation(out=h[:], in_=h[:], func=Act.Relu)
                        nc.sync.dma_start(out=ag_in[t * P:(t + 1) * P, :], in_=h[:])
                    else:
                        if zero_b2:
                            nc.scalar.mul(out=h[:], in_=h_ps[:], mul=sc)
                        else:
                            nc.vector.tensor_scalar_mul(h[:], h_ps[:], sc)
                            nc.vector.tensor_tensor(out=h[:], in0=h[:], in1=b2_t[:],
                                                    op=Alu.add)
                        spt = sp.tile([P, GPC], f32, tag="sp")
                        nc.sync.dma_start(out=spt[:],
                                          in_=spool_in[t * P:(t + 1) * P, :])
                        nc.tensor.matmul(out=pool_ps[:], lhsT=h[:], rhs=spt[:],
                                         start=(t == 0), stop=(t == TILES - 1))
                return pool_ps

            layer(x_g[:], w1_t, True)
            if variant == "noag":
                for c8 in range(NC):
                    nc.sync.dma_start(out=h1g[c8 * NPC:(c8 + 1) * NPC, :],
                                      in_=ag_in[:])
            else:
                nc.gpsimd.collective_compute(
                    "AllGather", Alu.bypass, replica_groups=[list(range(NC))],
                    ins=[ag_in[:]], outs=[h1g[:]],
                )
            pool_ps = layer(h1g[:], w2_t, False)

            # ---- head: gT = pooled mean [128 h, 256 g] ----
            wih_t = io.tile([D, 3 * D], f32)
            brz_t = io.tile([P, 2], f32)
            bn_t = io.tile([P, 2], f32)
            wlin_t = io.tile([D, 1], f32)
            blin_t = io.tile([1, 1], f32)
            ones_m = io.tile([P, 1], f32)   # 1/128 for mean matmuls
            eps_t = io.tile([1, 1], f32)
            ones_r = io.tile([1, P], f32)   # row of ones for bcast matmuls
            nc.sync.dma_start(out=wih_t[:], in_=wih_in[:])
            nc.sync.dma_start(out=brz_t[:], in_=bias_rz_in[:])
            nc.sync.dma_start(out=bn_t[:], in_=bias_n_in[:])
            nc.sync.dma_start(out=wlin_t[:], in_=wlin_in[:])
            nc.sync.dma_start(out=blin_t[:], in_=blin_in[:])
            nc.vector.memset(ones_m[:], 1.0 / P)
            nc.vector.memset(eps_t[:], EPS)
            nc.vector.memset(ones_r[:], 1.0)

            hd = wk.tile([P, GPC], f32, tag="hd")
            nc.scalar.copy(out=hd[:], in_=pool_ps[:])  # gT [h, 256]

            def gate_mm(sl):
                ps = ps_h.tile([P, GPC], f32, space="PSUM", tag="hps")
                nc.tensor.matmul(out=ps[:], lhsT=wih_t[:, sl * D:(sl + 1) * D],
                                 rhs=hd[:], start=True, stop=True)
                return ps

            r = wk.tile([P, GPC], f32, tag="r")
            nc.scalar.activation(out=r[:], in_=gate_mm(0)[:], func=Act.Sigmoid,
                                 bias=brz_t[:, 0:1], scale=1.0)
            zz = wk.tile([P, GPC], f32, tag="zz")
            nc.scalar.activation(out=zz[:], in_=gate_mm(1)[:], func=Act.Sigmoid,
                                 bias=brz_t[:, 1:2], scale=1.0)
            nps = gate_mm(2)
            rb = wk.tile([P, GPC], f32, tag="rb")
            nc.vector.tensor_scalar_mul(rb[:], r[:], bn_t[:, 1:2])
            t1 = wk.tile([P, GPC], f32, tag="t1")
            nc.vector.tensor_tensor(out=t1[:], in0=nps[:], in1=rb[:], op=Alu.add)
            n_t = wk.tile([P, GPC], f32, tag="nt")
            nc.scalar.activation(out=n_t[:], in_=t1[:], func=Act.Tanh,
                                 bias=bn_t[:, 0:1], scale=1.0)
            zn = wk.tile([P, GPC], f32, tag="zn")
            nc.vector.tensor_tensor(out=zn[:], in0=zz[:], in1=n_t[:], op=Alu.mult)
            hr = wk.tile([P, GPC], f32, tag="hr")
            nc.vector.tensor_tensor(out=hr[:], in0=n_t[:], in1=zn[:],
                                    op=Alu.subtract)
            nc.scalar.activation(out=hr[:], in_=hr[:], func=Act.Relu)

            mu_ps = ps_h.tile([1, GPC], f32, space="PSUM", tag="hps")
            nc.tensor.matmul(out=mu_ps[:], lhsT=ones_m[:], rhs=hr[:],
                             start=True, stop=True)
            mu = wk.tile([1, GPC], f32, tag="mu")
            nc.scalar.copy(out=mu[:], in_=mu_ps[:])
            mub_ps = ps_h.tile([P, GPC], f32, space="PSUM", tag="hps")
            nc.tensor.matmul(out=mub_ps[:], lhsT=ones_r[:], rhs=mu[:],
                             start=True, stop=True)
            dmu = wk.tile([P, GPC], f32, tag="dmu")
            nc.vector.tensor_tensor(out=dmu[:], in0=hr[:], in1=mub_ps[:],
                                    op=Alu.subtract)
            d2 = wk.tile([P, GPC], f32, tag="d2")
            nc.scalar.activation(out=d2[:], in_=dmu[:], func=Act.Square)
            var_ps = ps_h.tile([1, GPC], f32, space="PSUM", tag="hps")
            nc.tensor.matmul(out=var_ps[:], lhsT=ones_m[:], rhs=d2[:],
                             start=True, stop=True)
            std = wk.tile([1, GPC], f32, tag="std")
            nc.scalar.activation(out=std[:], in_=var_ps[:], func=Act.Sqrt,
                                 bias=eps_t[:, 0:1])
            rstd = wk.tile([1, GPC], f32, tag="rstd")
            nc.vector.reciprocal(rstd[:], std[:])
            rsb_ps = ps_h.tile([P, GPC], f32, space="PSUM", tag="hps")
            nc.tensor.matmul(out=rsb_ps[:], lhsT=ones_r[:], rhs=rstd[:],
                             start=True, stop=True)
            gn = wk.tile([P, GPC], f32, tag="gn")
            nc.vector.tensor_tensor(out=gn[:], in0=dmu[:], in1=rsb_ps[:],
                                    op=Alu.mult)
            o_ps = ps_h.tile([1, GPC], f32, space="PSUM", tag="hps")
            nc.tensor.matmul(out=o_ps[:], lhsT=wlin_t[:], rhs=gn[:],
                             start=True, stop=True)
            o_sb = wk.tile([1, GPC], f32, tag="o")
            nc.scalar.activation(out=o_sb[:], in_=o_ps[:], func=Act.Identity,
                                 bias=blin_t[:, 0:1], scale=1.0)
            nc.sync.dma_start(out=out[:], in_=o_sb[:])

    nc.compile()
    return nc


def kernel(**inputs):
    x = np.ascontiguousarray(np.asarray(inputs["x"], dtype=np.float32))
    ei = np.asarray(inputs["edge_index"]).astype(np.int64)
    batch = np.asarray(inputs["batch"]).astype(np.int64)
    W1 = np.asarray(inputs["W1"], np.float32)
    b1 = np.asarray(inputs["b1"], np.float32)
    W2 = np.asarray(inputs["W2"], np.float32)
    b2 = np.asarray(inputs["b2"], np.float32)
    W_ih = np.asarray(inputs["W_ih"], np.float32)
    W_hh = np.asarray(inputs["W_hh"], np.float32)
    b_ih = np.asarray(inputs["b_ih"], np.float32)
    b_hh = np.asarray(inputs["b_hh"], np.float32)
    W_lin = np.asarray(inputs["W_lin"], np.float32)
    b_lin = np.asarray(inputs["b_lin"], np.float32)
    del W_hh  # unused: h0 == 0 makes gh = b_hh

    prep = _prep(x, ei[0], ei[1], batch)

    zero_b1 = not np.any(b1)
    zero_b2 = not np.any(b2)
    key = (prep["ktot"], zero_b1, zero_b2)
    if key not in _CACHE:
        _CACHE[key] = _build(prep["ksched"], prep["tile_base"],
                             prep["bank_base"], prep["ktot"], zero_b1, zero_b2)
    nc = _CACHE[key]

    # gate weight blocks as lhsT: [h, fo] per gate (r, z, n)
    wih = np.concatenate([W_ih[i * D:(i + 1) * D, :].T for i in range(3)],
                         axis=1).astype(np.float32)  # [128, 384]
    bias_rz = np.stack([b_ih[0:D] + b_hh[0:D], b_ih[D:2 * D] + b_hh[D:2 * D]],
                       axis=1).astype(np.float32)  # [128, 2]
    bias_n = np.stack([b_ih[2 * D:], b_hh[2 * D:]], axis=1).astype(np.float32)
    b1b = np.tile(b1[None, :], (P, 1)).astype(np.float32)
    b2b = np.tile(b2[None, :], (P, 1)).astype(np.float32)

    in_maps = []
    for c in range(NC):
        in_maps.append({
            "x_g": prep["x_g"],
            "idx": prep["idx"][c],
            "nrm": prep["nrm"][c],
            "dinvt": prep["dinvt"][c],
            "spool": prep["spool"][c],
            "w1": W1, "w2": W2,
            "wih": wih, "bias_rz": bias_rz, "bias_n": bias_n,
            "wlin": W_lin.T.astype(np.float32).reshape(D, 1),
            "blin": b_lin.reshape(1, 1).astype(np.float32),
            "b1b": b1b, "b2b": b2b,
        })

    global _last_in_maps
    _last_in_maps = in_maps
    from concourse.bass_utils import run_bass_kernel_spmd
    res = run_bass_kernel_spmd(nc, in_maps, core_ids=list(range(NC)))
    out = np.concatenate([res.results[c]["out"][0] for c in range(NC)])
    return out.reshape(N_GRAPHS, 1).astype(np.float32)
